# revision 13
# baseline (speedup 1.0000x reference)
"""MLA decode paged attention (flat_pa_mla latent-cache path) on 8 TRN2 NeuronCores.

Sharding: data-parallel over the block/batch axis. Blocks are grouped 16-per-request
(asserted), so each core gets 4 complete requests = 64 blocks and computes its slice
of the output independently — no collectives.

Host prep per core (the "paged per device" part of the sharding) pre-swizzles the
gathered pages into SBUF-tile order so every bulk DMA is a contiguous
[128 partitions x 8KB] transfer:
  ktl [4, 2, 128, 4096]: per (request, group-pair) K^T lora tiles — free axis is
      (group 2, d-chunk 4, 512 block-positions) of kv[block].T.
  ktr [4, 2, 65, 1024]: rope rows 512..575 plus a block_bias row, so the bias
      addition folds into the QK matmul against a constant-1.0 row in qt.
  vh  [8, 128, 4096]: natural-layout value pages, free axis (position-pair 2,
      request 4, 512 latent) for one ~1MB DMA per block-position pair.
  qt  [4, 577, 16]: per-request SCALE*query transposed, with a trailing 1.0 row.

Device (per core), 4 requests in lockstep at 32-partition stride so element-wise
work runs on 128 partitions and the 4 per-request matmuls run concurrently in
separate PE column groups (tile_position):
  pass A per group: per request 5 PE matmuls (lhsT = qt chunk [<=128,16], rhs =
  [<=128,512]) accumulate attn in pa[32r:32r+16, :] of one PSUM bank [128,512];
  one DVE per-block max, one ACT exp(bias=-max) per block, one DVE per-group sum.
  One combine: grouped max/sum -> rescale [128,16]. Pass B per block position:
  scale p [128,128], PE-transpose to [128,128], 4 PV matmuls (lhsT = p^T columns
  32r..32r+16, rhs = v page) accumulating the [128,512] output PSUM bank.
  Dummy matmuls on zeros warm the PE clock gate during the DMA head and the
  pass A -> pass B combine latency.
"""

import numpy as np

import concourse.bass as bass
import concourse.mybir as mybir
import concourse.tile as tile
from concourse import bacc
from concourse.bass_utils import run_bass_kernel_spmd
from concourse.masks import make_identity

B = 32
H = 16
KVL = 512
ROPE = 64
D = KVL + ROPE          # 576
BS = 128
BPS = 16                # blocks per request
NB = B * BPS            # 512
SCALE = 192 ** -0.5
NCORES = 8
RPC = B // NCORES       # 4 requests per core
NBLK = RPC * BPS        # 64 blocks per core
BPG = 4                 # blocks per qk-group (one N=512 matmul)
NGR = BPS // BPG        # 4 qk-groups per request
NPAIR = NGR // 2        # group-pairs per request (one ~1MB kt DMA each)
DR = D + 1              # 577 rows: 576 latent+rope dims + 1 bias row
RR = DR - 512           # 65 rope+bias rows
RST = 32                # per-request partition stride (PE col groups are 32-wide)
HP = RPC * RST          # 128 partitions spanned by packed per-request ops

KV_DT = mybir.dt.bfloat16
P_DT = mybir.dt.bfloat16

TRACE = False           # set True (with profhook installed) to NTFF-profile
LAST_RESULTS = None     # BassKernelResults of the last kernel() call when TRACE

_NC_CACHE = {}


def _np_of(dt):
    import ml_dtypes

    return {mybir.dt.float32: np.float32, mybir.dt.bfloat16: ml_dtypes.bfloat16}[dt]


def _build(kv_dt, p_dt):
    f32 = mybir.dt.float32
    nc = bacc.Bacc("TRN2", target_bir_lowering=False, debug=False)
    ktl = nc.dram_tensor(
        "ktl", [RPC, NPAIR, 128, 2 * 4 * BPG * BS], kv_dt, kind="ExternalInput"
    ).ap()
    ktr = nc.dram_tensor(
        "ktr", [RPC, NPAIR, RR, 2 * BPG * BS], kv_dt, kind="ExternalInput"
    ).ap()
    vh = nc.dram_tensor(
        "vh", [BPS // 2, BS, 2 * RPC * KVL], kv_dt, kind="ExternalInput"
    ).ap()
    qt = nc.dram_tensor("qt", [RPC, DR, H], kv_dt, kind="ExternalInput").ap()
    o = nc.dram_tensor("o", [RPC, H, KVL], f32, kind="ExternalOutput").ap()

    with tile.TileContext(nc) as tc:
        with (
            tc.tile_pool(name="singles", bufs=1) as singles,
            tc.tile_pool(name="ktp", bufs=2) as ktp,
            tc.tile_pool(name="krp", bufs=2) as krp,
            tc.tile_pool(name="vp", bufs=4) as vp,
            tc.tile_pool(name="pp", bufs=4) as pp,
            tc.tile_pool(name="stats", bufs=10) as stats,
            tc.tile_pool(name="pap", bufs=3, space="PSUM") as pap,
            tc.tile_pool(name="ptpp", bufs=3, space="PSUM") as ptpp,
            tc.tile_pool(name="pop", bufs=1, space="PSUM") as pop,
            tc.tile_pool(name="warm", bufs=1, space="PSUM") as warmp,
        ):
            # PE warm-up: no input deps, runs during the DMA head and flips the
            # HAM clock gate before the real matmuls arrive.
            wz = singles.tile([128, 512], kv_dt)
            nc.vector.memset(wz, 0.0)
            warm_ps = warmp.tile([128, 512], f32)
            for k in range(18):
                h = 256 * (k % 2)
                nc.tensor.matmul(warm_ps[:, h : h + 256], wz[:, 0:128], wz[:, 0:256])

            # kt DMAs: contiguous ~1MB (lora) + ~133KB (rope+bias) per
            # (request, group-pair); issue split across the two HWDGE rings.
            klt = {}
            krt = {}
            for ip in range(NPAIR):
                for r in range(RPC):
                    eng = nc.sync if r < 2 else nc.scalar
                    kl = ktp.tile([128, 2, 4, BPG * BS], kv_dt, tag=f"kl{r}")
                    eng.dma_start(
                        out=kl,
                        in_=ktl[r, ip].rearrange("p (g c s) -> p g c s", g=2, c=4),
                    )
                    kr = krp.tile([RR, 2, BPG * BS], kv_dt, tag=f"kr{r}")
                    eng.dma_start(
                        out=kr, in_=ktr[r, ip].rearrange("p (g s) -> p g s", g=2)
                    )
                    klt[(2 * ip, r)] = (kl, 0)
                    klt[(2 * ip + 1, r)] = (kl, 1)
                    krt[(2 * ip, r)] = (kr, 0)
                    krt[(2 * ip + 1, r)] = (kr, 1)

            qt1 = singles.tile([128, RPC, 4, H], kv_dt)
            qt2 = singles.tile([RR, RPC, H], kv_dt)
            for r in range(RPC):
                nc.gpsimd.dma_start(
                    out=qt1[:, r, :, :],
                    in_=qt[r, 0 : 4 * 128, :].rearrange("(c p) h -> p c h", p=128),
                )
                nc.gpsimd.dma_start(out=qt2[:, r, :], in_=qt[r, 512:DR, :])

            ident = singles.tile([HP, HP], p_dt)
            make_identity(nc, ident)

            p_all = singles.tile([HP, BPS, BS], p_dt)
            bm = stats.tile([HP, BPS], f32)
            nbm = stats.tile([HP, BPS], f32)
            sums = stats.tile([HP, BPS], f32)

            # ---- pass A: QK + per-block softmax stats ----
            for i in range(NGR):
                pa = pap.tile([HP, BPG * BS], f32)
                for c in range(4):
                    for r in range(RPC):
                        kl, g = klt[(i, r)]
                        nc.tensor.matmul(
                            pa[RST * r : RST * r + H, :],
                            qt1[:, r, c, :],
                            kl[:, g, c, :],
                            start=(c == 0),
                            stop=False,
                            tile_position=(0, RST * r),
                        )
                for r in range(RPC):
                    kr, g = krt[(i, r)]
                    nc.tensor.matmul(
                        pa[RST * r : RST * r + H, :],
                        qt2[:, r, :],
                        kr[:, g, :],
                        start=False,
                        stop=True,
                        tile_position=(0, RST * r),
                    )

                bsl = slice(BPG * i, BPG * (i + 1))
                nc.vector.reduce_max(
                    out=bm[:, bsl],
                    in_=pa.rearrange("h (j s) -> h j s", j=BPG),
                    axis=mybir.AxisListType.X,
                )
                nc.vector.tensor_scalar_mul(nbm[:, bsl], bm[:, bsl], -1.0)
                for j in range(BPG):
                    idx = BPG * i + j
                    nc.scalar.activation(
                        out=p_all[:, idx, :],
                        in_=pa[:, BS * j : BS * (j + 1)],
                        func=mybir.ActivationFunctionType.Exp,
                        bias=nbm[:, idx : idx + 1],
                        scale=1.0,
                    )
                nc.vector.reduce_sum(
                    out=sums[:, bsl],
                    in_=p_all[:, BPG * i : BPG * (i + 1), :],
                    axis=mybir.AxisListType.X,
                )

            # bridge: keep the PE busy across the combine latency
            for k in range(14):
                h = 256 * (k % 2)
                nc.tensor.matmul(warm_ps[:, h : h + 256], wz[:, 0:128], wz[:, 0:256])

            # ---- combine: grouped max/sum -> rescale [HP, BPS] ----
            gm = stats.tile([HP, 1], f32)
            ngm = stats.tile([HP, 1], f32)
            adj = stats.tile([HP, BPS], f32)
            sa = stats.tile([HP, BPS], f32)
            gs = stats.tile([HP, 1], f32)
            gsm = stats.tile([HP, BPS], f32)
            rgs = stats.tile([HP, BPS], f32)
            resc = stats.tile([HP, BPS], f32)
            nc.vector.reduce_max(out=gm, in_=bm, axis=mybir.AxisListType.X)
            nc.vector.tensor_scalar_mul(ngm, gm, -1.0)
            nc.scalar.activation(
                out=adj,
                in_=bm,
                func=mybir.ActivationFunctionType.Exp,
                bias=ngm[:, 0:1],
                scale=1.0,
            )
            nc.vector.tensor_mul(sa, sums, adj)
            nc.vector.reduce_sum(out=gs, in_=sa, axis=mybir.AxisListType.X)
            nc.vector.tensor_scalar_max(gsm, sa, gs[:, 0:1])
            nc.vector.reciprocal(rgs, gsm)
            nc.vector.tensor_mul(resc, adj, rgs)

            # ---- pass B: rescale p, transpose, PV accumulate ----
            po = pop.tile([HP, KVL], f32)
            vtiles = {}
            for idx in range(BPS):
                if idx % 2 == 0:
                    vtile = vp.tile([128, 2, RPC, KVL], kv_dt)
                    veng = nc.sync if idx % 4 == 0 else nc.scalar
                    veng.dma_start(
                        out=vtile,
                        in_=vh[idx // 2].rearrange(
                            "s (g r e) -> s g r e", g=2, r=RPC
                        ),
                    )
                    vtiles[idx] = (vtile, 0)
                    vtiles[idx + 1] = (vtile, 1)
                ps = pp.tile([HP, BS], p_dt)
                nc.vector.tensor_scalar_mul(
                    ps, p_all[:, idx, :], resc[:, idx : idx + 1]
                )
                ptp = ptpp.tile([BS, HP], p_dt)
                nc.tensor.transpose(ptp, ps, ident)
                pt_sb = pp.tile([BS, HP], kv_dt)
                nc.vector.tensor_copy(pt_sb, ptp)
                vt, g = vtiles[idx]
                for r in range(RPC):
                    nc.tensor.matmul(
                        po[RST * r : RST * r + H, :],
                        pt_sb[:, RST * r : RST * r + H],
                        vt[:, g, r, :],
                        start=(idx == 0),
                        stop=(idx == BPS - 1),
                        tile_position=(0, RST * r),
                    )
            o_sb = singles.tile([HP, KVL], f32)
            nc.scalar.copy(o_sb, po)
            for r in range(RPC):
                oeng = nc.sync if r % 2 == 0 else nc.scalar
                oeng.dma_start(out=o[r], in_=o_sb[RST * r : RST * r + H, :])

    nc.compile()
    return nc


def _get_nc():
    key = (KV_DT, P_DT)
    if key not in _NC_CACHE:
        _NC_CACHE[key] = _build(*key)
    return _NC_CACHE[key]


def kernel(query, key_cache, block_mapping, block_bias, block_list, block_groups):
    global LAST_RESULTS
    query = np.asarray(query)
    key_cache = np.asarray(key_cache)
    block_bias = np.asarray(block_bias)
    block_list = np.asarray(block_list)
    block_groups = np.asarray(block_groups)

    # Sort blocks by request; each request must own exactly BPS blocks.
    perm = np.argsort(block_groups, kind="stable")
    bg = block_groups[perm]
    assert (np.bincount(bg, minlength=B) == BPS).all()
    bl = block_list[perm]
    bias = block_bias[perm].astype(np.float32)

    np_kv = _np_of(KV_DT)
    pages = key_cache[bl]  # [NB, BS, D] gathered pages ("paged per device")

    nc = _get_nc()
    in_maps = []
    for cc in range(NCORES):
        sl = slice(NBLK * cc, NBLK * (cc + 1))
        pg = np.asarray(pages[sl], dtype=np_kv)  # [64, 128, 576]
        pgT = pg.transpose(0, 2, 1)              # [64, 576, 128]
        # lora rows -> [r, ip, p, (g, c, j, b)]
        lora = pgT[:, :KVL, :].reshape(RPC, NPAIR, 2, BPG, 4, 128, BS)
        ktl = np.ascontiguousarray(lora.transpose(0, 1, 5, 2, 4, 3, 6)).reshape(
            RPC, NPAIR, 128, 2 * 4 * BPG * BS
        )
        # rope rows + bias row -> [r, ip, p, (g, j, b)]
        rb = np.concatenate(
            [pgT[:, KVL:, :], bias[sl].astype(np_kv).reshape(NBLK, 1, BS)], axis=1
        )  # [64, 65, 128]
        rb = rb.reshape(RPC, NPAIR, 2, BPG, RR, BS)
        ktr = np.ascontiguousarray(rb.transpose(0, 1, 4, 2, 3, 5)).reshape(
            RPC, NPAIR, RR, 2 * BPG * BS
        )
        # v pages -> [ipp, s, (g, r, e)]
        vv = pg[:, :, :KVL].reshape(RPC, BPS // 2, 2, BS, KVL)
        vhh = np.ascontiguousarray(vv.transpose(1, 3, 2, 0, 4)).reshape(
            BPS // 2, BS, 2 * RPC * KVL
        )
        qtt = np.empty((RPC, DR, H), np_kv)
        qtt[:, :D, :] = (SCALE * query[RPC * cc : RPC * (cc + 1)]).transpose(0, 2, 1)
        qtt[:, D, :] = 1.0
        in_maps.append({"ktl": ktl, "ktr": ktr, "vh": vhh, "qt": qtt})

    res = run_bass_kernel_spmd(nc, in_maps, list(range(NCORES)), trace=TRACE)
    if TRACE:
        LAST_RESULTS = res
    return np.concatenate(
        [res.results[i]["o"] for i in range(NCORES)], axis=0
    ).astype(np.float32)
